# revision 10
# baseline (speedup 1.0000x reference)
"""Distributed GQA attention kernel for 8 TRN2 NeuronCores.

Sharding (tensor-parallel over heads): core i owns q-heads [8i, 8i+8) and
kv-head i (GQA n_rep=8, so one kv head serves all 8 local q heads). Each core:
  1. QKV projection from the full x (weights pre-transposed host-side),
     computed in f32r (full-rate fp32-rounded matmuls).
  2. RoPE on qT/kT in [d, s] layout (sin staged sign-folded).
  3. Causal attention per head in transposed-score layout [k, q]:
     exp(scale*s) with no max subtraction (scores are O(6)), the attention
     sink enters as +exp(sink) in the denominator, and denominators ride an
     extra ones-column appended to v.
  4. Local slice of the output projection -> partial yT [2880, 1536].
  5. ReduceScatter(add) over the 8 cores; core i gets yT rows [360i, 360i+360),
     adds the wo bias slice. Host concatenates/transposes shards.
"""

import contextlib
import ctypes
import os
import sys

import numpy as np

sys.path.insert(0, "/opt/trn_rl_repo")

S = 1536
HID = 2880
D = 64
HL = 8          # local q heads per core
CORES = 8
SCQ = 512       # QKV moving chunk
NSC = S // SCQ
QC = 512        # attention q chunk
NQC = S // QC
KBN = S // 128  # 12 k blocks
VA = 68         # v_aug padded width (f32r moving dim must be 4-aligned)
CBF = 22        # full 128-row contraction blocks (2880 = 22*128 + 64)
MBN = 23        # wo output row blocks (22 full + one 64)
JBN = 4         # 512 local j rows = 4 blocks

_EXEC_TIME_NS = [None]


def _install_hooks():
    import types

    import antenv

    try:
        from antenv import axon_hooks
    except ImportError:
        axon_hooks = types.ModuleType("antenv.axon_hooks")
        _holder = {"hook": None}
        axon_hooks.set_axon_ntff_profile_hook = lambda h: _holder.update(hook=h)
        axon_hooks.get_axon_ntff_profile_hook = lambda: _holder["hook"]
        sys.modules["antenv.axon_hooks"] = axon_hooks
        antenv.axon_hooks = axon_hooks

    so_path = "/opt/axon/libaxon_pjrt.so"
    hook = None
    if os.path.exists(so_path):
        lib = ctypes.CDLL(so_path)
        if hasattr(lib, "axon_start_nrt_profile"):
            lib.axon_start_nrt_profile.argtypes = [
                ctypes.POINTER(ctypes.c_int64),
                ctypes.c_size_t,
            ]
            lib.axon_start_nrt_profile.restype = ctypes.c_int64
            lib.axon_stop_nrt_profile.argtypes = [ctypes.c_char_p]
            lib.axon_stop_nrt_profile.restype = ctypes.c_int64

            @contextlib.contextmanager
            def hook(output_dir, device_ids):
                import jax

                jax.devices()
                if device_ids:
                    ids = (ctypes.c_int64 * len(device_ids))(*device_ids)
                    rc = lib.axon_start_nrt_profile(ids, len(device_ids))
                else:
                    rc = lib.axon_start_nrt_profile(None, 0)
                if rc != 0:
                    raise RuntimeError(f"axon_start_nrt_profile rc={rc}")
                try:
                    yield
                finally:
                    n = lib.axon_stop_nrt_profile(str(output_dir).encode())
                    print(f"profile: {n} file(s) written to {output_dir}")

    axon_hooks.set_axon_ntff_profile_hook(hook)

    import concourse.bass_utils as bu

    bu.upload_artifacts = lambda tmpdir: f"local://{tmpdir}"


def build_graph():
    import concourse.mybir as mybir
    import concourse.tile as tile
    from concourse import bacc
    from concourse.masks import make_identity

    F32 = mybir.dt.float32
    F32R = mybir.dt.float32r
    BF16 = mybir.dt.bfloat16

    nc = bacc.Bacc("TRN2", target_bir_lowering=False, debug=False, num_devices=CORES)

    xT = nc.declare_dram_parameter("xT", [HID, S], BF16, isOutput=False)
    wT = nc.declare_dram_parameter("wT", [HID, 640], BF16, isOutput=False)
    bq = nc.declare_dram_parameter("bq", [64, HL], F32, isOutput=False)
    bk = nc.declare_dram_parameter("bk", [64, 1], F32, isOutput=False)
    bv = nc.declare_dram_parameter("bv", [64, 1], F32, isOutput=False)
    cosT = nc.declare_dram_parameter("cosT", [64, S], F32, isOutput=False)
    sinTs = nc.declare_dram_parameter("sinTs", [64, S], F32, isOutput=False)
    woT = nc.declare_dram_parameter("woT", [512, HID], BF16, isOutput=False)
    wob = nc.declare_dram_parameter("wob", [128, 3], F32, isOutput=False)
    esink = nc.declare_dram_parameter("esink", [128, HL], F32, isOutput=False)
    out = nc.declare_dram_parameter("out", [360, S], F32, isOutput=True)

    yT_part = [
        nc.dram_tensor(f"yT_part{c}", [HID, QC], F32) for c in range(NQC)
    ]
    yT_red = [
        nc.dram_tensor(f"yT_red{c}", [360, QC], F32) for c in range(NQC)
    ]

    Exp = mybir.ActivationFunctionType.Exp

    with tile.TileContext(nc) as tc:
        with contextlib.ExitStack() as stack:
            consts = stack.enter_context(tc.tile_pool(name="consts", bufs=1))
            qkvout = stack.enter_context(tc.tile_pool(name="qkvout", bufs=1))
            small = stack.enter_context(tc.tile_pool(name="small", bufs=4))
            ytp = stack.enter_context(tc.tile_pool(name="ytp", bufs=3))

            bqt = consts.tile([64, HL], F32, tag="bq")
            bkt = consts.tile([64, 1], F32, tag="bk")
            bvt = consts.tile([64, 1], F32, tag="bv")
            cost = consts.tile([64, S], F32, tag="cos")
            sint = consts.tile([64, S], F32, tag="sin")
            wobt = consts.tile([128, 3], F32, tag="wob")
            esk = consts.tile([128, HL], F32, tag="esk")
            ident_f = consts.tile([128, 128], F32, tag="ident_f")
            ident = consts.tile([128, 128], BF16, tag="ident")
            ones = consts.tile([128, 1], F32, tag="ones")
            for t, src in [(bqt, bq), (bkt, bk), (bvt, bv), (cost, cosT),
                           (sint, sinTs), (wobt, wob), (esk, esink)]:
                nc.sync.dma_start(out=t[:, :], in_=src[:, :])
            make_identity(nc, ident_f[:, :])
            nc.vector.tensor_copy(ident[:, :], ident_f[:, :])
            nc.vector.memset(ones[:, :], 1.0)

            # head-major rotated q [64, HL*S], k [64, S], v^T [64, S], v_aug [128, 12*65]
            qq = qkvout.tile([64, HL * S], BF16, tag="qq")
            kh = qkvout.tile([64, S], BF16, tag="kh")
            vT = qkvout.tile([64, S], BF16, tag="vT")
            vaug = qkvout.tile([128, KBN * VA], BF16, tag="vaug")

            # ---------------- QKV projection ----------------
            with (
                tc.tile_pool(name="wtp", bufs=1) as wtp,
                tc.tile_pool(name="xcp", bufs=2) as xcp,
                tc.tile_pool(name="qkps", bufs=4, space="PSUM") as qkps,
                tc.tile_pool(name="rtmp", bufs=3) as rtmp,
            ):
                wtt = wtp.tile([128, CBF * 640], BF16, tag="wtt")
                wt2 = wtp.tile([64, 640], BF16, tag="wt2")
                nc.sync.dma_start(
                    out=wtt[:, :].rearrange("p (cb n) -> p cb n", cb=CBF),
                    in_=wT[0 : CBF * 128, :].rearrange("(cb p) n -> p cb n", p=128),
                )
                nc.sync.dma_start(out=wt2[:, :], in_=wT[CBF * 128 : HID, :])

                for sc in range(NSC):
                    c0 = sc * SCQ
                    xc = xcp.tile([128, CBF * SCQ], BF16, tag="xc")
                    xc2 = xcp.tile([64, SCQ], BF16, tag="xc2")
                    nc.sync.dma_start(
                        out=xc[:, :].rearrange("p (cb s) -> p cb s", cb=CBF),
                        in_=xT[0 : CBF * 128, c0 : c0 + SCQ].rearrange(
                            "(cb p) s -> p cb s", p=128
                        ),
                    )
                    nc.sync.dma_start(
                        out=xc2[:, :], in_=xT[CBF * 128 : HID, c0 : c0 + SCQ]
                    )
                    for nb in range(5):
                        p = qkps.tile([128, SCQ], F32, tag="qkv")
                        for cb in range(CBF):
                            nc.tensor.matmul(
                                p[:, :],
                                wtt[:, cb * 640 + nb * 128 : cb * 640 + (nb + 1) * 128],
                                xc[:, cb * SCQ : (cb + 1) * SCQ],
                                start=(cb == 0),
                                stop=False,
                            )
                        nc.tensor.matmul(
                            p[:, :],
                            wt2[:, nb * 128 : (nb + 1) * 128],
                            xc2[:, :],
                            start=False,
                            stop=True,
                        )
                        if nb < 4:
                            for half in range(2):
                                h = 2 * nb + half
                                hb = 64 * half
                                qb = rtmp.tile([64, SCQ], F32, tag="qb")
                                nc.vector.tensor_scalar_add(
                                    qb[:, :], p[hb : hb + 64, :], bqt[:, h : h + 1]
                                )
                                rot = rtmp.tile([64, SCQ], F32, tag="rot")
                                nc.scalar.copy(rot[0:32, :], qb[32:64, :])
                                nc.scalar.copy(rot[32:64, :], qb[0:32, :])
                                nc.vector.tensor_mul(
                                    qb[:, :], qb[:, :], cost[:, c0 : c0 + SCQ]
                                )
                                nc.vector.tensor_mul(
                                    rot[:, :], rot[:, :], sint[:, c0 : c0 + SCQ]
                                )
                                nc.vector.tensor_add(
                                    qq[:, h * S + c0 : h * S + c0 + SCQ],
                                    qb[:, :],
                                    rot[:, :],
                                )
                        else:
                            kb_ = rtmp.tile([64, SCQ], F32, tag="qb")
                            nc.vector.tensor_scalar_add(
                                kb_[:, :], p[0:64, :], bkt[:, 0:1]
                            )
                            rot = rtmp.tile([64, SCQ], F32, tag="rot")
                            nc.scalar.copy(rot[0:32, :], kb_[32:64, :])
                            nc.scalar.copy(rot[32:64, :], kb_[0:32, :])
                            nc.vector.tensor_mul(
                                kb_[:, :], kb_[:, :], cost[:, c0 : c0 + SCQ]
                            )
                            nc.vector.tensor_mul(
                                rot[:, :], rot[:, :], sint[:, c0 : c0 + SCQ]
                            )
                            nc.vector.tensor_add(
                                kh[:, c0 : c0 + SCQ], kb_[:, :], rot[:, :]
                            )
                            nc.vector.tensor_scalar_add(
                                vT[:, c0 : c0 + SCQ], p[64:128, :], bvt[:, 0:1]
                            )

            # ---------------- v transpose + ones column ----------------
            with tc.tile_pool(name="vtp", bufs=2, space="PSUM") as vtp:
                for kb in range(KBN):
                    pv = vtp.tile([128, D], BF16, tag="pv")
                    nc.tensor.transpose(
                        pv[:, :], vT[:, kb * 128 : (kb + 1) * 128], ident[0:64, 0:64]
                    )
                    nc.vector.tensor_copy(
                        vaug[:, kb * VA : kb * VA + 64], pv[:, :]
                    )
                    for oc in range(64, VA):
                        nc.vector.tensor_copy(
                            vaug[:, kb * VA + oc : kb * VA + oc + 1], ones[:, :]
                        )

            # ---------------- attention + wo ----------------
            with (
                tc.tile_pool(name="oTp", bufs=1) as oTp,
                tc.tile_pool(name="woTp", bufs=1) as woTp,
                tc.tile_pool(name="esp", bufs=6) as esp,
                tc.tile_pool(name="scps", bufs=3, space="PSUM") as scps,
                tc.tile_pool(name="pops", bufs=2, space="PSUM") as pops,
                tc.tile_pool(name="wops", bufs=2, space="PSUM") as wops,
            ):
                oTt = oTp.tile([128, JBN * S], BF16, tag="oT")
                woTt = woTp.tile([128, JBN * HID], BF16, tag="woT")
                nc.sync.dma_start(
                    out=woTt[:, :].rearrange("p (jb m) -> p jb m", jb=JBN),
                    in_=woT[:, :].rearrange("(jb p) m -> p jb m", p=128),
                )
                for qc in range(NQC):
                    q0 = qc * QC
                    qb0 = q0 // 128
                    for h in range(HL):
                        nkb = qb0 + 4
                        es_tiles = [None] * nkb
                        po = pops.tile([VA, QC], F32, tag="po")

                        def emit_scores(kb):
                            ps_s = scps.tile([128, QC], F32, tag="scores")
                            nc.tensor.matmul(
                                ps_s[:, :],
                                kh[:, kb * 128 : (kb + 1) * 128],
                                qq[:, h * S + q0 : h * S + q0 + QC],
                                start=True,
                                stop=True,
                            )
                            es = esp.tile([128, QC], BF16, tag="es")
                            nc.scalar.activation(es[:, :], ps_s[:, :], Exp, scale=0.125)
                            if kb >= qb0:
                                nc.gpsimd.affine_select(
                                    out=es[:, :],
                                    in_=es[:, :],
                                    compare_op=mybir.AluOpType.is_ge,
                                    fill=0.0,
                                    base=q0 - kb * 128,
                                    pattern=[[1, QC]],
                                    channel_multiplier=-1,
                                )
                            es_tiles[kb] = es

                        def emit_av(kb):
                            nc.tensor.matmul(
                                po[:, :],
                                vaug[:, kb * VA : kb * VA + VA],
                                es_tiles[kb][:, :],
                                start=(kb == 0),
                                stop=(kb == nkb - 1),
                            )

                        # lag-2 software pipeline: AV(kb-2) after scores(kb)
                        for kb in range(nkb):
                            emit_scores(kb)
                            if kb >= 2:
                                emit_av(kb - 2)
                        emit_av(nkb - 2)
                        emit_av(nkb - 1)

                        o_u = esp.tile([VA, QC], F32, tag="o_u")
                        nc.vector.tensor_copy(o_u[:, :], po[:, :])
                        for s in range(4):
                            ptf = scps.tile([128, VA], F32, tag="scores")
                            nc.tensor.transpose(
                                ptf[:, :],
                                o_u[:, s * 128 : (s + 1) * 128],
                                ident_f[0:VA, 0:VA],
                            )
                            denom = small.tile([128, 1], F32, tag="denom")
                            nc.vector.tensor_add(
                                denom[:, :], ptf[:, D : D + 1], esk[:, h : h + 1]
                            )
                            recip = small.tile([128, 1], F32, tag="recip")
                            nc.vector.reciprocal(recip[:, :], denom[:, :])
                            o_n = small.tile([128, D], BF16, tag="o_n")
                            nc.vector.tensor_scalar_mul(
                                o_n[:, :], ptf[:, 0:D], recip[:, :]
                            )
                            pt = scps.tile([64, 128], BF16, tag="scores")
                            nc.tensor.transpose(pt[:, :], o_n[:, :], ident[:, :])
                            jb, ro = h // 2, (h % 2) * 64
                            nc.vector.tensor_copy(
                                oTt[
                                    ro : ro + 64,
                                    jb * S + q0 + s * 128 : jb * S + q0 + (s + 1) * 128,
                                ],
                                pt[:, :],
                            )
                    # wo for this s-chunk
                    for mb in range(MBN):
                        rows = 128 if mb < CBF else 64
                        pw = wops.tile([128, QC], F32, tag="wo")
                        for jb in range(JBN):
                            nc.tensor.matmul(
                                pw[0:rows, :],
                                woTt[:, jb * HID + mb * 128 : jb * HID + mb * 128 + rows],
                                oTt[:, jb * S + q0 : jb * S + q0 + QC],
                                start=(jb == 0),
                                stop=(jb == JBN - 1),
                            )
                        yt = ytp.tile([128, QC], F32, tag="yt")
                        nc.vector.tensor_copy(yt[0:rows, :], pw[0:rows, :])
                        nc.sync.dma_start(
                            out=yT_part[qc][mb * 128 : mb * 128 + rows, :],
                            in_=yt[0:rows, :],
                        )
                        if mb == 10:
                            nc.gpsimd.collective_compute(
                                "ReduceScatter",
                                mybir.AluOpType.add,
                                replica_groups=[list(range(CORES))],
                                ins=[yT_part[qc][0:1408, :].opt()],
                                outs=[yT_red[qc][0:176, :].opt()],
                            )
                    nc.gpsimd.collective_compute(
                        "ReduceScatter",
                        mybir.AluOpType.add,
                        replica_groups=[list(range(CORES))],
                        ins=[yT_part[qc][1408:HID, :].opt()],
                        outs=[yT_red[qc][176:360, :].opt()],
                    )

            # ---------------- bias tail (RS issued per-qc above) ----------------
            for t in range(3):
                rows = 128 if t < 2 else 104
                for cc in range(NQC):
                    rt = ytp.tile([128, QC], F32, tag="yt")
                    nc.sync.dma_start(
                        out=rt[0:rows, :],
                        in_=yT_red[cc][t * 128 : t * 128 + rows, :],
                    )
                    nc.vector.tensor_scalar_add(
                        rt[0:rows, :], rt[0:rows, :], wobt[0:rows, t : t + 1]
                    )
                    nc.sync.dma_start(
                        out=out[t * 128 : t * 128 + rows, cc * QC : (cc + 1) * QC],
                        in_=rt[0:rows, :],
                    )

    nc.finalize()
    return nc


def make_in_maps(x, rope_cache, wq_w, wq_b, wk_w, wk_b, wv_w, wv_b, wo_w, wo_b, sinks):
    import ml_dtypes

    BF = ml_dtypes.bfloat16
    xT = np.ascontiguousarray(x[0].T).astype(BF)  # [2880, 1536]
    cosT = np.ascontiguousarray(rope_cache[:, :D].T, dtype=np.float32)
    sinT = np.ascontiguousarray(rope_cache[:, D:].T, dtype=np.float32)
    sinTs = sinT.copy()
    sinTs[: D // 2] *= -1.0

    in_maps = []
    for i in range(CORES):
        wq = wq_w[512 * i : 512 * (i + 1)]
        wk = wk_w[64 * i : 64 * (i + 1)]
        wv = wv_w[64 * i : 64 * (i + 1)]
        wT = np.ascontiguousarray(np.concatenate([wq, wk, wv], axis=0).T).astype(BF)
        bq = np.ascontiguousarray(
            wq_b[512 * i : 512 * (i + 1)].reshape(HL, 64).T, np.float32
        )
        bk = wk_b[64 * i : 64 * (i + 1)].reshape(64, 1).astype(np.float32)
        bv = wv_b[64 * i : 64 * (i + 1)].reshape(64, 1).astype(np.float32)
        woT = np.ascontiguousarray(wo_w[:, 512 * i : 512 * (i + 1)].T).astype(BF)
        seg = np.concatenate(
            [
                wo_b[176 * i : 176 * (i + 1)],
                wo_b[1408 + 184 * i : 1408 + 184 * (i + 1)],
            ]
        )
        wob = np.zeros((128, 3), np.float32)
        for t in range(3):
            piece = seg[128 * t : 128 * (t + 1)]
            wob[: len(piece), t] = piece
        es = np.exp(sinks[HL * i : HL * (i + 1)]).astype(np.float32)
        esink = np.repeat(es[None, :], 128, axis=0).astype(np.float32)
        in_maps.append(
            {
                "xT": xT,
                "wT": wT,
                "bq": bq,
                "bk": bk,
                "bv": bv,
                "cosT": cosT,
                "sinTs": sinTs,
                "woT": woT,
                "wob": np.ascontiguousarray(wob),
                "esink": esink,
            }
        )
    return in_maps


_CACHE = {}


def kernel(**inputs):
    _install_hooks()
    from concourse import bass_utils

    trace = bool(int(os.environ.get("BASS_KERNEL_TRACE", "0")))
    if "nc" not in _CACHE:
        _CACHE["nc"] = build_graph()
    nc = _CACHE["nc"]

    in_maps = make_in_maps(**{k: np.asarray(v) for k, v in inputs.items()})
    res = bass_utils.run_bass_kernel_spmd(
        nc, in_maps, core_ids=list(range(CORES)), trace=trace
    )
    _EXEC_TIME_NS[0] = res.exec_time_ns

    y = np.empty((S, HID), np.float32)
    for i in range(CORES):
        o = res.results[i]["out"]
        y[:, 176 * i : 176 * (i + 1)] = o[0:176].T
        y[:, 1408 + 184 * i : 1408 + 184 * (i + 1)] = o[176:360].T
    return y.reshape(1, S, HID)


def last_exec_time_ns():
    return _EXEC_TIME_NS[0]


# revision 12
# speedup vs baseline: 1.1805x; 1.1805x over previous
"""Distributed GQA attention kernel for 8 TRN2 NeuronCores.

Sharding (tensor-parallel over heads): core i owns q-heads [8i, 8i+8) and
kv-head i (GQA n_rep=8, so one kv head serves all 8 local q heads). Each core:
  1. QKV projection from the full x (weights pre-transposed host-side),
     computed in f32r (full-rate fp32-rounded matmuls).
  2. RoPE on qT/kT in [d, s] layout (sin staged sign-folded).
  3. Causal attention per head in transposed-score layout [k, q]:
     exp(scale*s) with no max subtraction (scores are O(6)), the attention
     sink enters as +exp(sink) in the denominator, and denominators ride an
     extra ones-column appended to v.
  4. Local slice of the output projection -> partial yT [2880, 1536].
  5. ReduceScatter(add) over the 8 cores; core i gets yT rows [360i, 360i+360),
     adds the wo bias slice. Host concatenates/transposes shards.
"""

import contextlib
import ctypes
import os
import sys

import numpy as np

sys.path.insert(0, "/opt/trn_rl_repo")

S = 1536
HID = 2880
D = 64
HL = 8          # local q heads per core
CORES = 8
SCQ = 512       # QKV moving chunk
NSC = S // SCQ
QC = 512        # attention q chunk
NQC = S // QC
KBN = S // 128  # 12 k blocks
VA = 68         # v_aug padded width (f32r moving dim must be 4-aligned)
CBF = 22        # full 128-row contraction blocks (2880 = 22*128 + 64)
MBN = 23        # wo output row blocks (22 full + one 64)
JBN = 4         # 512 local j rows = 4 blocks

_EXEC_TIME_NS = [None]


def _install_hooks():
    import types

    import antenv

    try:
        from antenv import axon_hooks
    except ImportError:
        axon_hooks = types.ModuleType("antenv.axon_hooks")
        _holder = {"hook": None}
        axon_hooks.set_axon_ntff_profile_hook = lambda h: _holder.update(hook=h)
        axon_hooks.get_axon_ntff_profile_hook = lambda: _holder["hook"]
        sys.modules["antenv.axon_hooks"] = axon_hooks
        antenv.axon_hooks = axon_hooks

    so_path = "/opt/axon/libaxon_pjrt.so"
    hook = None
    if os.path.exists(so_path):
        lib = ctypes.CDLL(so_path)
        if hasattr(lib, "axon_start_nrt_profile"):
            lib.axon_start_nrt_profile.argtypes = [
                ctypes.POINTER(ctypes.c_int64),
                ctypes.c_size_t,
            ]
            lib.axon_start_nrt_profile.restype = ctypes.c_int64
            lib.axon_stop_nrt_profile.argtypes = [ctypes.c_char_p]
            lib.axon_stop_nrt_profile.restype = ctypes.c_int64

            @contextlib.contextmanager
            def hook(output_dir, device_ids):
                import jax

                jax.devices()
                if device_ids:
                    ids = (ctypes.c_int64 * len(device_ids))(*device_ids)
                    rc = lib.axon_start_nrt_profile(ids, len(device_ids))
                else:
                    rc = lib.axon_start_nrt_profile(None, 0)
                if rc != 0:
                    raise RuntimeError(f"axon_start_nrt_profile rc={rc}")
                try:
                    yield
                finally:
                    n = lib.axon_stop_nrt_profile(str(output_dir).encode())
                    print(f"profile: {n} file(s) written to {output_dir}")

    axon_hooks.set_axon_ntff_profile_hook(hook)

    import concourse.bass_utils as bu

    bu.upload_artifacts = lambda tmpdir: f"local://{tmpdir}"


def build_graph():
    import concourse.mybir as mybir
    import concourse.tile as tile
    from concourse import bacc
    from concourse.masks import make_identity

    F32 = mybir.dt.float32
    F32R = mybir.dt.float32r
    BF16 = mybir.dt.bfloat16

    nc = bacc.Bacc("TRN2", target_bir_lowering=False, debug=False, num_devices=CORES)

    xT = nc.declare_dram_parameter("xT", [HID, S], BF16, isOutput=False)
    wT = nc.declare_dram_parameter("wT", [HID, 640], BF16, isOutput=False)
    bq = nc.declare_dram_parameter("bq", [64, HL], F32, isOutput=False)
    bk = nc.declare_dram_parameter("bk", [64, 1], F32, isOutput=False)
    bv = nc.declare_dram_parameter("bv", [64, 1], F32, isOutput=False)
    cosT = nc.declare_dram_parameter("cosT", [64, S], F32, isOutput=False)
    sinTs = nc.declare_dram_parameter("sinTs", [64, S], F32, isOutput=False)
    woT = nc.declare_dram_parameter("woT", [512, HID], BF16, isOutput=False)
    wob = nc.declare_dram_parameter("wob", [128, 3], F32, isOutput=False)
    esink = nc.declare_dram_parameter("esink", [128, HL], F32, isOutput=False)
    out = nc.declare_dram_parameter("out", [360, S], F32, isOutput=True)

    yT_part = [
        nc.dram_tensor(f"yT_part{c}", [HID, QC], F32) for c in range(NQC)
    ]
    yT_red = [
        nc.dram_tensor(f"yT_red{c}", [360, QC], F32) for c in range(NQC)
    ]

    Exp = mybir.ActivationFunctionType.Exp

    with tile.TileContext(nc) as tc:
        with contextlib.ExitStack() as stack:
            consts = stack.enter_context(tc.tile_pool(name="consts", bufs=1))
            qkvout = stack.enter_context(tc.tile_pool(name="qkvout", bufs=1))
            small = stack.enter_context(tc.tile_pool(name="small", bufs=4))
            ytp = stack.enter_context(tc.tile_pool(name="ytp", bufs=3))

            bqt = consts.tile([64, HL], F32, tag="bq")
            bkt = consts.tile([64, 1], F32, tag="bk")
            bvt = consts.tile([64, 1], F32, tag="bv")
            cost = consts.tile([64, S], F32, tag="cos")
            sint = consts.tile([64, S], F32, tag="sin")
            wobt = consts.tile([128, 3], F32, tag="wob")
            esk = consts.tile([128, HL], F32, tag="esk")
            ident_f = consts.tile([128, 128], F32, tag="ident_f")
            ident = consts.tile([128, 128], BF16, tag="ident")
            ones = consts.tile([128, 1], F32, tag="ones")
            for t, src in [(bqt, bq), (bkt, bk), (bvt, bv), (cost, cosT),
                           (sint, sinTs), (wobt, wob), (esk, esink)]:
                nc.sync.dma_start(out=t[:, :], in_=src[:, :])
            make_identity(nc, ident_f[:, :])
            nc.vector.tensor_copy(ident[:, :], ident_f[:, :])
            nc.vector.memset(ones[:, :], 1.0)
            trimasks = []
            for j in range(4):
                tm = consts.tile([128, QC], BF16, tag=f"tri{j}")
                nc.vector.memset(tm[:, :], 1.0)
                nc.gpsimd.affine_select(
                    out=tm[:, :],
                    in_=tm[:, :],
                    compare_op=mybir.AluOpType.is_ge,
                    fill=0.0,
                    base=-128 * j,
                    pattern=[[1, QC]],
                    channel_multiplier=-1,
                )
                trimasks.append(tm)

            # head-major rotated q [64, HL*S], k [64, S], v^T [64, S], v_aug [128, 12*65]
            qq = qkvout.tile([64, HL * S], BF16, tag="qq")
            kh = qkvout.tile([64, S], BF16, tag="kh")
            vT = qkvout.tile([64, S], BF16, tag="vT")
            vaug = qkvout.tile([128, KBN * VA], BF16, tag="vaug")

            # ---------------- QKV projection ----------------
            with (
                tc.tile_pool(name="wtp", bufs=1) as wtp,
                tc.tile_pool(name="xcp", bufs=2) as xcp,
                tc.tile_pool(name="qkps", bufs=4, space="PSUM") as qkps,
                tc.tile_pool(name="rtmp", bufs=3) as rtmp,
            ):
                wtt = wtp.tile([128, CBF * 640], BF16, tag="wtt")
                wt2 = wtp.tile([64, 640], BF16, tag="wt2")
                nc.sync.dma_start(
                    out=wtt[:, :].rearrange("p (cb n) -> p cb n", cb=CBF),
                    in_=wT[0 : CBF * 128, :].rearrange("(cb p) n -> p cb n", p=128),
                )
                nc.sync.dma_start(out=wt2[:, :], in_=wT[CBF * 128 : HID, :])

                for sc in range(NSC):
                    c0 = sc * SCQ
                    xc = xcp.tile([128, CBF * SCQ], BF16, tag="xc")
                    xc2 = xcp.tile([64, SCQ], BF16, tag="xc2")
                    nc.sync.dma_start(
                        out=xc[:, :].rearrange("p (cb s) -> p cb s", cb=CBF),
                        in_=xT[0 : CBF * 128, c0 : c0 + SCQ].rearrange(
                            "(cb p) s -> p cb s", p=128
                        ),
                    )
                    nc.sync.dma_start(
                        out=xc2[:, :], in_=xT[CBF * 128 : HID, c0 : c0 + SCQ]
                    )
                    for nb in range(5):
                        p = qkps.tile([128, SCQ], F32, tag="qkv")
                        for cb in range(CBF):
                            nc.tensor.matmul(
                                p[:, :],
                                wtt[:, cb * 640 + nb * 128 : cb * 640 + (nb + 1) * 128],
                                xc[:, cb * SCQ : (cb + 1) * SCQ],
                                start=(cb == 0),
                                stop=False,
                            )
                        nc.tensor.matmul(
                            p[:, :],
                            wt2[:, nb * 128 : (nb + 1) * 128],
                            xc2[:, :],
                            start=False,
                            stop=True,
                        )
                        if nb < 4:
                            for half in range(2):
                                h = 2 * nb + half
                                hb = 64 * half
                                qb = rtmp.tile([64, SCQ], F32, tag="qb")
                                nc.vector.tensor_scalar_add(
                                    qb[:, :], p[hb : hb + 64, :], bqt[:, h : h + 1]
                                )
                                rot = rtmp.tile([64, SCQ], F32, tag="rot")
                                nc.scalar.copy(rot[0:32, :], qb[32:64, :])
                                nc.scalar.copy(rot[32:64, :], qb[0:32, :])
                                nc.vector.tensor_mul(
                                    qb[:, :], qb[:, :], cost[:, c0 : c0 + SCQ]
                                )
                                nc.vector.tensor_mul(
                                    rot[:, :], rot[:, :], sint[:, c0 : c0 + SCQ]
                                )
                                nc.vector.tensor_add(
                                    qq[:, h * S + c0 : h * S + c0 + SCQ],
                                    qb[:, :],
                                    rot[:, :],
                                )
                        else:
                            kb_ = rtmp.tile([64, SCQ], F32, tag="qb")
                            nc.vector.tensor_scalar_add(
                                kb_[:, :], p[0:64, :], bkt[:, 0:1]
                            )
                            rot = rtmp.tile([64, SCQ], F32, tag="rot")
                            nc.scalar.copy(rot[0:32, :], kb_[32:64, :])
                            nc.scalar.copy(rot[32:64, :], kb_[0:32, :])
                            nc.vector.tensor_mul(
                                kb_[:, :], kb_[:, :], cost[:, c0 : c0 + SCQ]
                            )
                            nc.vector.tensor_mul(
                                rot[:, :], rot[:, :], sint[:, c0 : c0 + SCQ]
                            )
                            nc.vector.tensor_add(
                                kh[:, c0 : c0 + SCQ], kb_[:, :], rot[:, :]
                            )
                            nc.vector.tensor_scalar_add(
                                vT[:, c0 : c0 + SCQ], p[64:128, :], bvt[:, 0:1]
                            )

            # ---------------- v transpose + ones column ----------------
            with tc.tile_pool(name="vtp", bufs=2, space="PSUM") as vtp:
                for kb in range(KBN):
                    pv = vtp.tile([128, D], BF16, tag="pv")
                    nc.tensor.transpose(
                        pv[:, :], vT[:, kb * 128 : (kb + 1) * 128], ident[0:64, 0:64]
                    )
                    nc.vector.tensor_copy(
                        vaug[:, kb * VA : kb * VA + 64], pv[:, :]
                    )
                    for oc in range(64, VA):
                        nc.vector.tensor_copy(
                            vaug[:, kb * VA + oc : kb * VA + oc + 1], ones[:, :]
                        )

            # ---------------- attention + wo ----------------
            with (
                tc.tile_pool(name="oTp", bufs=1) as oTp,
                tc.tile_pool(name="woTp", bufs=1) as woTp,
                tc.tile_pool(name="esp", bufs=6) as esp,
                tc.tile_pool(name="scps", bufs=2, space="PSUM") as scps,
                tc.tile_pool(name="pops", bufs=2, space="PSUM") as pops,
                tc.tile_pool(name="wops", bufs=3, space="PSUM") as wops,
            ):
                oTt = oTp.tile([128, JBN * S], BF16, tag="oT")
                woTt = woTp.tile([128, JBN * HID], BF16, tag="woT")
                nc.sync.dma_start(
                    out=woTt[:, :].rearrange("p (jb m) -> p jb m", jb=JBN),
                    in_=woT[:, :].rearrange("(jb p) m -> p jb m", p=128),
                )
                for qc in range(NQC):
                    q0 = qc * QC
                    qb0 = q0 // 128
                    for h in range(HL):
                        nkb = qb0 + 4
                        es_tiles = [None] * nkb
                        po = pops.tile([VA, QC], F32, tag="po")

                        def emit_scores(kb):
                            ps_s = scps.tile([128, QC], F32, tag="scores")
                            nc.tensor.matmul(
                                ps_s[:, :],
                                kh[:, kb * 128 : (kb + 1) * 128],
                                qq[:, h * S + q0 : h * S + q0 + QC],
                                start=True,
                                stop=True,
                            )
                            es = esp.tile([128, QC], BF16, tag="es")
                            nc.scalar.activation(es[:, :], ps_s[:, :], Exp, scale=0.125)
                            if kb >= qb0:
                                j = kb - qb0
                                if kb % 2 == 0:
                                    nc.gpsimd.affine_select(
                                        out=es[:, :],
                                        in_=es[:, :],
                                        compare_op=mybir.AluOpType.is_ge,
                                        fill=0.0,
                                        base=q0 - kb * 128,
                                        pattern=[[1, QC]],
                                        channel_multiplier=-1,
                                    )
                                else:
                                    nc.vector.tensor_mul(
                                        es[:, :], es[:, :], trimasks[j][:, :]
                                    )
                            es_tiles[kb] = es

                        def emit_av(kb):
                            nc.tensor.matmul(
                                po[:, :],
                                vaug[:, kb * VA : kb * VA + VA],
                                es_tiles[kb][:, :],
                                start=(kb == 0),
                                stop=(kb == nkb - 1),
                            )

                        # lag-2 software pipeline: AV(kb-2) after scores(kb)
                        for kb in range(nkb):
                            emit_scores(kb)
                            if kb >= 2:
                                emit_av(kb - 2)
                        emit_av(nkb - 2)
                        emit_av(nkb - 1)

                        o_u = esp.tile([VA, QC], F32, tag="o_u")
                        nc.vector.tensor_copy(o_u[:, :], po[:, :])
                        for s in range(4):
                            ptf = scps.tile([128, VA], F32, tag="scores")
                            nc.tensor.transpose(
                                ptf[:, :],
                                o_u[:, s * 128 : (s + 1) * 128],
                                ident_f[0:VA, 0:VA],
                            )
                            denom = small.tile([128, 1], F32, tag="denom")
                            nc.vector.tensor_add(
                                denom[:, :], ptf[:, D : D + 1], esk[:, h : h + 1]
                            )
                            recip = small.tile([128, 1], F32, tag="recip")
                            nc.vector.reciprocal(recip[:, :], denom[:, :])
                            o_n = small.tile([128, D], BF16, tag="o_n")
                            nc.vector.tensor_scalar_mul(
                                o_n[:, :], ptf[:, 0:D], recip[:, :]
                            )
                            pt = scps.tile([64, 128], BF16, tag="scores")
                            nc.tensor.transpose(pt[:, :], o_n[:, :], ident[:, :])
                            jb, ro = h // 2, (h % 2) * 64
                            dst = oTt[
                                ro : ro + 64,
                                jb * S + q0 + s * 128 : jb * S + q0 + (s + 1) * 128,
                            ]
                            if s % 2 == 0:
                                nc.vector.tensor_copy(dst, pt[:, :])
                            else:
                                nc.scalar.copy(dst, pt[:, :])
                    # wo for this s-chunk
                    for mb in range(MBN):
                        rows = 128 if mb < CBF else 64
                        pw = wops.tile([128, QC], F32, tag="wo")
                        for jb in range(JBN):
                            nc.tensor.matmul(
                                pw[0:rows, :],
                                woTt[:, jb * HID + mb * 128 : jb * HID + mb * 128 + rows],
                                oTt[:, jb * S + q0 : jb * S + q0 + QC],
                                start=(jb == 0),
                                stop=(jb == JBN - 1),
                            )
                        yt = ytp.tile([128, QC], F32, tag="yt")
                        if mb % 2 == 0:
                            nc.vector.tensor_copy(yt[0:rows, :], pw[0:rows, :])
                        else:
                            nc.scalar.copy(yt[0:rows, :], pw[0:rows, :])
                        nc.sync.dma_start(
                            out=yT_part[qc][mb * 128 : mb * 128 + rows, :],
                            in_=yt[0:rows, :],
                        )
                    nc.gpsimd.collective_compute(
                        "ReduceScatter",
                        mybir.AluOpType.add,
                        replica_groups=[list(range(CORES))],
                        ins=[yT_part[qc].ap().opt()],
                        outs=[yT_red[qc].ap().opt()],
                    )

            # ---------------- bias tail (RS issued per-qc above) ----------------
            for t in range(3):
                rows = 128 if t < 2 else 104
                for cc in range(NQC):
                    rt = ytp.tile([128, QC], F32, tag="yt")
                    nc.sync.dma_start(
                        out=rt[0:rows, :],
                        in_=yT_red[cc][t * 128 : t * 128 + rows, :],
                    )
                    nc.vector.tensor_scalar_add(
                        rt[0:rows, :], rt[0:rows, :], wobt[0:rows, t : t + 1]
                    )
                    nc.sync.dma_start(
                        out=out[t * 128 : t * 128 + rows, cc * QC : (cc + 1) * QC],
                        in_=rt[0:rows, :],
                    )

    nc.finalize()
    return nc


def make_in_maps(x, rope_cache, wq_w, wq_b, wk_w, wk_b, wv_w, wv_b, wo_w, wo_b, sinks):
    import ml_dtypes

    BF = ml_dtypes.bfloat16
    xT = np.ascontiguousarray(x[0].T).astype(BF)  # [2880, 1536]
    cosT = np.ascontiguousarray(rope_cache[:, :D].T, dtype=np.float32)
    sinT = np.ascontiguousarray(rope_cache[:, D:].T, dtype=np.float32)
    sinTs = sinT.copy()
    sinTs[: D // 2] *= -1.0

    in_maps = []
    for i in range(CORES):
        wq = wq_w[512 * i : 512 * (i + 1)]
        wk = wk_w[64 * i : 64 * (i + 1)]
        wv = wv_w[64 * i : 64 * (i + 1)]
        wT = np.ascontiguousarray(np.concatenate([wq, wk, wv], axis=0).T).astype(BF)
        bq = np.ascontiguousarray(
            wq_b[512 * i : 512 * (i + 1)].reshape(HL, 64).T, np.float32
        )
        bk = wk_b[64 * i : 64 * (i + 1)].reshape(64, 1).astype(np.float32)
        bv = wv_b[64 * i : 64 * (i + 1)].reshape(64, 1).astype(np.float32)
        woT = np.ascontiguousarray(wo_w[:, 512 * i : 512 * (i + 1)].T).astype(BF)
        seg = wo_b[360 * i : 360 * (i + 1)]
        wob = np.zeros((128, 3), np.float32)
        for t in range(3):
            piece = seg[128 * t : 128 * (t + 1)]
            wob[: len(piece), t] = piece
        es = np.exp(sinks[HL * i : HL * (i + 1)]).astype(np.float32)
        esink = np.repeat(es[None, :], 128, axis=0).astype(np.float32)
        in_maps.append(
            {
                "xT": xT,
                "wT": wT,
                "bq": bq,
                "bk": bk,
                "bv": bv,
                "cosT": cosT,
                "sinTs": sinTs,
                "woT": woT,
                "wob": np.ascontiguousarray(wob),
                "esink": esink,
            }
        )
    return in_maps


_CACHE = {}


def kernel(**inputs):
    _install_hooks()
    from concourse import bass_utils

    trace = bool(int(os.environ.get("BASS_KERNEL_TRACE", "0")))
    if "nc" not in _CACHE:
        _CACHE["nc"] = build_graph()
    nc = _CACHE["nc"]

    in_maps = make_in_maps(**{k: np.asarray(v) for k, v in inputs.items()})
    res = bass_utils.run_bass_kernel_spmd(
        nc, in_maps, core_ids=list(range(CORES)), trace=trace
    )
    _EXEC_TIME_NS[0] = res.exec_time_ns

    y = np.empty((S, HID), np.float32)
    for i in range(CORES):
        y[:, 360 * i : 360 * (i + 1)] = res.results[i]["out"].T
    return y.reshape(1, S, HID)


def last_exec_time_ns():
    return _EXEC_TIME_NS[0]


# revision 14
# speedup vs baseline: 1.5051x; 1.2750x over previous
"""Distributed GQA attention kernel for 8 TRN2 NeuronCores.

Sharding (tensor-parallel over heads): core i owns q-heads [8i, 8i+8) and
kv-head i (GQA n_rep=8, so one kv head serves all 8 local q heads). Each core:
  1. QKV projection from the full x (weights pre-transposed host-side),
     computed in f32r (full-rate fp32-rounded matmuls).
  2. RoPE on qT/kT in [d, s] layout (sin staged sign-folded).
  3. Causal attention per head in transposed-score layout [k, q]:
     exp(scale*s) with no max subtraction (scores are O(6)), the attention
     sink enters as +exp(sink) in the denominator, and denominators ride an
     extra ones-column appended to v.
  4. Local slice of the output projection -> partial yT [2880, 1536].
  5. ReduceScatter(add) over the 8 cores; core i gets yT rows [360i, 360i+360),
     adds the wo bias slice. Host concatenates/transposes shards.
"""

import contextlib
import ctypes
import os
import sys

import numpy as np

sys.path.insert(0, "/opt/trn_rl_repo")

S = 1536
HID = 2880
D = 64
HL = 8          # local q heads per core
CORES = 8
SCQ = 512       # QKV moving chunk
NSC = S // SCQ
QC = 512        # attention q chunk
NQC = S // QC
KBN = S // 128  # 12 k blocks
VA = 68         # v_aug padded width (f32r moving dim must be 4-aligned)
CBF = 22        # full 128-row contraction blocks (2880 = 22*128 + 64)
MBN = 23        # wo output row blocks (22 full + one 64)
JBN = 4         # 512 local j rows = 4 blocks

_EXEC_TIME_NS = [None]


def _install_hooks():
    import types

    import antenv

    try:
        from antenv import axon_hooks
    except ImportError:
        axon_hooks = types.ModuleType("antenv.axon_hooks")
        _holder = {"hook": None}
        axon_hooks.set_axon_ntff_profile_hook = lambda h: _holder.update(hook=h)
        axon_hooks.get_axon_ntff_profile_hook = lambda: _holder["hook"]
        sys.modules["antenv.axon_hooks"] = axon_hooks
        antenv.axon_hooks = axon_hooks

    so_path = "/opt/axon/libaxon_pjrt.so"
    hook = None
    if os.path.exists(so_path):
        lib = ctypes.CDLL(so_path)
        if hasattr(lib, "axon_start_nrt_profile"):
            lib.axon_start_nrt_profile.argtypes = [
                ctypes.POINTER(ctypes.c_int64),
                ctypes.c_size_t,
            ]
            lib.axon_start_nrt_profile.restype = ctypes.c_int64
            lib.axon_stop_nrt_profile.argtypes = [ctypes.c_char_p]
            lib.axon_stop_nrt_profile.restype = ctypes.c_int64

            @contextlib.contextmanager
            def hook(output_dir, device_ids):
                import jax

                jax.devices()
                if device_ids:
                    ids = (ctypes.c_int64 * len(device_ids))(*device_ids)
                    rc = lib.axon_start_nrt_profile(ids, len(device_ids))
                else:
                    rc = lib.axon_start_nrt_profile(None, 0)
                if rc != 0:
                    raise RuntimeError(f"axon_start_nrt_profile rc={rc}")
                try:
                    yield
                finally:
                    n = lib.axon_stop_nrt_profile(str(output_dir).encode())
                    print(f"profile: {n} file(s) written to {output_dir}")

    axon_hooks.set_axon_ntff_profile_hook(hook)

    import concourse.bass_utils as bu

    bu.upload_artifacts = lambda tmpdir: f"local://{tmpdir}"


def build_graph():
    import concourse.mybir as mybir
    import concourse.tile as tile
    from concourse import bacc
    from concourse.masks import make_identity

    F32 = mybir.dt.float32
    F32R = mybir.dt.float32r
    BF16 = mybir.dt.bfloat16

    nc = bacc.Bacc("TRN2", target_bir_lowering=False, debug=False, num_devices=CORES)

    xT = nc.declare_dram_parameter("xT", [HID, S], BF16, isOutput=False)
    wT = nc.declare_dram_parameter("wT", [HID, 640], BF16, isOutput=False)
    bq = nc.declare_dram_parameter("bq", [64, HL], F32, isOutput=False)
    bk = nc.declare_dram_parameter("bk", [64, 1], F32, isOutput=False)
    bv = nc.declare_dram_parameter("bv", [64, 1], F32, isOutput=False)
    cosT = nc.declare_dram_parameter("cosT", [64, S], F32, isOutput=False)
    sinTs = nc.declare_dram_parameter("sinTs", [64, S], F32, isOutput=False)
    woT = nc.declare_dram_parameter("woT", [512, HID], BF16, isOutput=False)
    wob = nc.declare_dram_parameter("wob", [128, 3], F32, isOutput=False)
    esink = nc.declare_dram_parameter("esink", [128, HL], F32, isOutput=False)
    out = nc.declare_dram_parameter("out", [360, S], F32, isOutput=True)

    yT_part = [
        nc.dram_tensor(f"yT_part{c}", [HID, QC], F32) for c in range(NQC)
    ]
    yT_red = [
        nc.dram_tensor(f"yT_red{c}", [360, QC], F32) for c in range(NQC)
    ]

    Exp = mybir.ActivationFunctionType.Exp

    with tile.TileContext(nc) as tc:
        with contextlib.ExitStack() as stack:
            consts = stack.enter_context(tc.tile_pool(name="consts", bufs=1))
            qkvout = stack.enter_context(tc.tile_pool(name="qkvout", bufs=1))
            small = stack.enter_context(tc.tile_pool(name="small", bufs=4))
            ytp = stack.enter_context(tc.tile_pool(name="ytp", bufs=3))

            bqt = consts.tile([64, HL], F32, tag="bq")
            bkt = consts.tile([64, 1], F32, tag="bk")
            bvt = consts.tile([64, 1], F32, tag="bv")
            cost = consts.tile([64, S], F32, tag="cos")
            sint = consts.tile([64, S], F32, tag="sin")
            wobt = consts.tile([128, 3], F32, tag="wob")
            esk = consts.tile([128, HL], F32, tag="esk")
            ident_f = consts.tile([128, 128], F32, tag="ident_f")
            ident = consts.tile([128, 128], BF16, tag="ident")
            ones = consts.tile([128, 1], F32, tag="ones")
            for t, src in [(bqt, bq), (bkt, bk), (bvt, bv), (cost, cosT),
                           (sint, sinTs), (wobt, wob), (esk, esink)]:
                nc.sync.dma_start(out=t[:, :], in_=src[:, :])
            make_identity(nc, ident_f[:, :])
            nc.vector.tensor_copy(ident[:, :], ident_f[:, :])
            nc.vector.memset(ones[:, :], 1.0)
            tri = consts.tile([128, 128], BF16, tag="tri")
            nc.vector.memset(tri[:, :], 1.0)
            nc.gpsimd.affine_select(
                out=tri[:, :],
                in_=tri[:, :],
                compare_op=mybir.AluOpType.is_ge,
                fill=0.0,
                base=0,
                pattern=[[1, 128]],
                channel_multiplier=-1,
            )

            # head-major rotated q [64, HL*S], k [64, S], v^T [64, S], v_aug [128, 12*65]
            qq = qkvout.tile([64, HL * S], BF16, tag="qq")
            kh = qkvout.tile([64, S], BF16, tag="kh")
            vT = qkvout.tile([64, S], BF16, tag="vT")
            vaug = qkvout.tile([128, KBN * VA], BF16, tag="vaug")

            # ---------------- QKV projection ----------------
            with (
                tc.tile_pool(name="wtp", bufs=1) as wtp,
                tc.tile_pool(name="xcp", bufs=2) as xcp,
                tc.tile_pool(name="qkps", bufs=4, space="PSUM") as qkps,
                tc.tile_pool(name="rtmp", bufs=3) as rtmp,
            ):
                wtt = wtp.tile([128, CBF * 640], BF16, tag="wtt")
                wt2 = wtp.tile([64, 640], BF16, tag="wt2")
                nc.sync.dma_start(
                    out=wtt[:, :].rearrange("p (cb n) -> p cb n", cb=CBF),
                    in_=wT[0 : CBF * 128, :].rearrange("(cb p) n -> p cb n", p=128),
                )
                nc.sync.dma_start(out=wt2[:, :], in_=wT[CBF * 128 : HID, :])

                for sc in range(NSC):
                    c0 = sc * SCQ
                    xc = xcp.tile([128, CBF * SCQ], BF16, tag="xc")
                    xc2 = xcp.tile([64, SCQ], BF16, tag="xc2")
                    nc.sync.dma_start(
                        out=xc[:, :].rearrange("p (cb s) -> p cb s", cb=CBF),
                        in_=xT[0 : CBF * 128, c0 : c0 + SCQ].rearrange(
                            "(cb p) s -> p cb s", p=128
                        ),
                    )
                    nc.sync.dma_start(
                        out=xc2[:, :], in_=xT[CBF * 128 : HID, c0 : c0 + SCQ]
                    )
                    for nb in range(5):
                        p = qkps.tile([128, SCQ], F32, tag="qkv")
                        for cb in range(CBF):
                            nc.tensor.matmul(
                                p[:, :],
                                wtt[:, cb * 640 + nb * 128 : cb * 640 + (nb + 1) * 128],
                                xc[:, cb * SCQ : (cb + 1) * SCQ],
                                start=(cb == 0),
                                stop=False,
                            )
                        nc.tensor.matmul(
                            p[:, :],
                            wt2[:, nb * 128 : (nb + 1) * 128],
                            xc2[:, :],
                            start=False,
                            stop=True,
                        )
                        if nb < 4:
                            for half in range(2):
                                h = 2 * nb + half
                                hb = 64 * half
                                qb = rtmp.tile([64, SCQ], F32, tag="qb")
                                nc.vector.tensor_scalar_add(
                                    qb[:, :], p[hb : hb + 64, :], bqt[:, h : h + 1]
                                )
                                rot = rtmp.tile([64, SCQ], F32, tag="rot")
                                nc.scalar.copy(rot[0:32, :], qb[32:64, :])
                                nc.scalar.copy(rot[32:64, :], qb[0:32, :])
                                nc.vector.tensor_mul(
                                    qb[:, :], qb[:, :], cost[:, c0 : c0 + SCQ]
                                )
                                nc.vector.tensor_mul(
                                    rot[:, :], rot[:, :], sint[:, c0 : c0 + SCQ]
                                )
                                nc.vector.tensor_add(
                                    qq[:, h * S + c0 : h * S + c0 + SCQ],
                                    qb[:, :],
                                    rot[:, :],
                                )
                        else:
                            kb_ = rtmp.tile([64, SCQ], F32, tag="qb")
                            nc.vector.tensor_scalar_add(
                                kb_[:, :], p[0:64, :], bkt[:, 0:1]
                            )
                            rot = rtmp.tile([64, SCQ], F32, tag="rot")
                            nc.scalar.copy(rot[0:32, :], kb_[32:64, :])
                            nc.scalar.copy(rot[32:64, :], kb_[0:32, :])
                            nc.vector.tensor_mul(
                                kb_[:, :], kb_[:, :], cost[:, c0 : c0 + SCQ]
                            )
                            nc.vector.tensor_mul(
                                rot[:, :], rot[:, :], sint[:, c0 : c0 + SCQ]
                            )
                            nc.vector.tensor_add(
                                kh[:, c0 : c0 + SCQ], kb_[:, :], rot[:, :]
                            )
                            nc.vector.tensor_scalar_add(
                                vT[:, c0 : c0 + SCQ], p[64:128, :], bvt[:, 0:1]
                            )

            # ---------------- v transpose + ones column ----------------
            with tc.tile_pool(name="vtp", bufs=2, space="PSUM") as vtp:
                for kb in range(KBN):
                    pv = vtp.tile([128, D], BF16, tag="pv")
                    nc.tensor.transpose(
                        pv[:, :], vT[:, kb * 128 : (kb + 1) * 128], ident[0:64, 0:64]
                    )
                    nc.vector.tensor_copy(
                        vaug[:, kb * VA : kb * VA + 64], pv[:, :]
                    )
                    for oc in range(64, VA):
                        nc.vector.tensor_copy(
                            vaug[:, kb * VA + oc : kb * VA + oc + 1], ones[:, :]
                        )

            # ---------------- attention + wo ----------------
            with (
                tc.tile_pool(name="oTp", bufs=1) as oTp,
                tc.tile_pool(name="woTp", bufs=1) as woTp,
                tc.tile_pool(name="esp", bufs=6) as esp,
                tc.tile_pool(name="scps", bufs=2, space="PSUM") as scps,
                tc.tile_pool(name="pops", bufs=1, space="PSUM") as pops,
                tc.tile_pool(name="wops", bufs=2, space="PSUM") as wops,
            ):
                oTt = oTp.tile([128, JBN * S], BF16, tag="oT")
                woTt = woTp.tile([128, JBN * HID], BF16, tag="woT")
                nc.sync.dma_start(
                    out=woTt[:, :].rearrange("p (jb m) -> p jb m", jb=JBN),
                    in_=woT[:, :].rearrange("(jb p) m -> p jb m", p=128),
                )
                for qc in range(NQC):
                    q0 = qc * QC
                    qb0 = q0 // 128
                    for h in range(HL):
                        nkb = qb0 + 4
                        es_tiles = [None] * nkb
                        po = [
                            pops.tile([128, VA], F32, tag=f"po{s}", name=f"po{s}_{qc}_{h}")
                            for s in range(4)
                        ]

                        def emit_scores(kb):
                            j = kb - qb0
                            w0 = 128 * j if j > 0 else 0
                            ps_s = scps.tile([128, QC], F32, tag="scores")
                            nc.tensor.matmul(
                                ps_s[:, :],
                                kh[:, kb * 128 : (kb + 1) * 128],
                                qq[:, h * S + q0 : h * S + q0 + QC],
                                start=True,
                                stop=True,
                            )
                            es = esp.tile([128, QC], BF16, tag="es")
                            nc.scalar.activation(
                                es[:, w0:QC], ps_s[:, w0:QC], Exp, scale=0.125
                            )
                            if j >= 0:
                                sl = slice(w0, w0 + 128)
                                if kb % 2 == 0:
                                    nc.gpsimd.affine_select(
                                        out=es[:, sl],
                                        in_=es[:, sl],
                                        compare_op=mybir.AluOpType.is_ge,
                                        fill=0.0,
                                        base=0,
                                        pattern=[[1, 128]],
                                        channel_multiplier=-1,
                                    )
                                else:
                                    nc.vector.tensor_mul(
                                        es[:, sl], es[:, sl], tri[:, :]
                                    )
                            es_tiles[kb] = es

                        def emit_avs(kb):
                            for s in range(4):
                                if qb0 + s >= kb:
                                    nc.tensor.matmul(
                                        po[s][:, :],
                                        es_tiles[kb][:, s * 128 : (s + 1) * 128],
                                        vaug[:, kb * VA : (kb + 1) * VA],
                                        start=(kb == 0),
                                        stop=(kb == qb0 + s),
                                    )

                        for kb in range(nkb):
                            emit_scores(kb)
                            if kb >= 2:
                                emit_avs(kb - 2)
                        emit_avs(nkb - 2)
                        emit_avs(nkb - 1)

                        for s in range(4):
                            denom = small.tile([128, 1], F32, tag="denom")
                            nc.vector.tensor_add(
                                denom[:, :], po[s][:, D : D + 1], esk[:, h : h + 1]
                            )
                            recip = small.tile([128, 1], F32, tag="recip")
                            nc.vector.reciprocal(recip[:, :], denom[:, :])
                            o_n = small.tile([128, D], BF16, tag="o_n")
                            nc.vector.tensor_scalar_mul(
                                o_n[:, :], po[s][:, 0:D], recip[:, :]
                            )
                            pt = scps.tile([64, 128], BF16, tag="scores")
                            nc.tensor.transpose(pt[:, :], o_n[:, :], ident[:, :])
                            jb, ro = h // 2, (h % 2) * 64
                            dst = oTt[
                                ro : ro + 64,
                                jb * S + q0 + s * 128 : jb * S + q0 + (s + 1) * 128,
                            ]
                            if s % 2 == 0:
                                nc.vector.tensor_copy(dst, pt[:, :])
                            else:
                                nc.scalar.copy(dst, pt[:, :])
                    # wo for this s-chunk
                    for mb in range(MBN):
                        rows = 128 if mb < CBF else 64
                        pw = wops.tile([128, QC], F32, tag="wo")
                        for jb in range(JBN):
                            nc.tensor.matmul(
                                pw[0:rows, :],
                                woTt[:, jb * HID + mb * 128 : jb * HID + mb * 128 + rows],
                                oTt[:, jb * S + q0 : jb * S + q0 + QC],
                                start=(jb == 0),
                                stop=(jb == JBN - 1),
                            )
                        yt = ytp.tile([128, QC], F32, tag="yt")
                        if mb % 2 == 0:
                            nc.vector.tensor_copy(yt[0:rows, :], pw[0:rows, :])
                        else:
                            nc.scalar.copy(yt[0:rows, :], pw[0:rows, :])
                        nc.sync.dma_start(
                            out=yT_part[qc][mb * 128 : mb * 128 + rows, :],
                            in_=yt[0:rows, :],
                        )
                    nc.gpsimd.collective_compute(
                        "ReduceScatter",
                        mybir.AluOpType.add,
                        replica_groups=[list(range(CORES))],
                        ins=[yT_part[qc].ap().opt()],
                        outs=[yT_red[qc].ap().opt()],
                    )

            # ---------------- bias tail (RS issued per-qc above) ----------------
            for t in range(3):
                rows = 128 if t < 2 else 104
                for cc in range(NQC):
                    rt = ytp.tile([128, QC], F32, tag="yt")
                    nc.sync.dma_start(
                        out=rt[0:rows, :],
                        in_=yT_red[cc][t * 128 : t * 128 + rows, :],
                    )
                    nc.vector.tensor_scalar_add(
                        rt[0:rows, :], rt[0:rows, :], wobt[0:rows, t : t + 1]
                    )
                    nc.sync.dma_start(
                        out=out[t * 128 : t * 128 + rows, cc * QC : (cc + 1) * QC],
                        in_=rt[0:rows, :],
                    )

    nc.finalize()
    return nc


def make_in_maps(x, rope_cache, wq_w, wq_b, wk_w, wk_b, wv_w, wv_b, wo_w, wo_b, sinks):
    import ml_dtypes

    BF = ml_dtypes.bfloat16
    xT = np.ascontiguousarray(x[0].T).astype(BF)  # [2880, 1536]
    cosT = np.ascontiguousarray(rope_cache[:, :D].T, dtype=np.float32)
    sinT = np.ascontiguousarray(rope_cache[:, D:].T, dtype=np.float32)
    sinTs = sinT.copy()
    sinTs[: D // 2] *= -1.0

    in_maps = []
    for i in range(CORES):
        wq = wq_w[512 * i : 512 * (i + 1)]
        wk = wk_w[64 * i : 64 * (i + 1)]
        wv = wv_w[64 * i : 64 * (i + 1)]
        wT = np.ascontiguousarray(np.concatenate([wq, wk, wv], axis=0).T).astype(BF)
        bq = np.ascontiguousarray(
            wq_b[512 * i : 512 * (i + 1)].reshape(HL, 64).T, np.float32
        )
        bk = wk_b[64 * i : 64 * (i + 1)].reshape(64, 1).astype(np.float32)
        bv = wv_b[64 * i : 64 * (i + 1)].reshape(64, 1).astype(np.float32)
        woT = np.ascontiguousarray(wo_w[:, 512 * i : 512 * (i + 1)].T).astype(BF)
        seg = wo_b[360 * i : 360 * (i + 1)]
        wob = np.zeros((128, 3), np.float32)
        for t in range(3):
            piece = seg[128 * t : 128 * (t + 1)]
            wob[: len(piece), t] = piece
        es = np.exp(sinks[HL * i : HL * (i + 1)]).astype(np.float32)
        esink = np.repeat(es[None, :], 128, axis=0).astype(np.float32)
        in_maps.append(
            {
                "xT": xT,
                "wT": wT,
                "bq": bq,
                "bk": bk,
                "bv": bv,
                "cosT": cosT,
                "sinTs": sinTs,
                "woT": woT,
                "wob": np.ascontiguousarray(wob),
                "esink": esink,
            }
        )
    return in_maps


_CACHE = {}


def kernel(**inputs):
    _install_hooks()
    from concourse import bass_utils

    trace = bool(int(os.environ.get("BASS_KERNEL_TRACE", "0")))
    if "nc" not in _CACHE:
        _CACHE["nc"] = build_graph()
    nc = _CACHE["nc"]

    in_maps = make_in_maps(**{k: np.asarray(v) for k, v in inputs.items()})
    res = bass_utils.run_bass_kernel_spmd(
        nc, in_maps, core_ids=list(range(CORES)), trace=trace
    )
    _EXEC_TIME_NS[0] = res.exec_time_ns

    y = np.empty((S, HID), np.float32)
    for i in range(CORES):
        y[:, 360 * i : 360 * (i + 1)] = res.results[i]["out"].T
    return y.reshape(1, S, HID)


def last_exec_time_ns():
    return _EXEC_TIME_NS[0]
